# revision 1
# baseline (speedup 1.0000x reference)
"""Trainium2 Bass kernel for nn_CrossAttention: two-stream (rgb/depth) cross
attention, B=8 batch elements data-parallel across 8 NeuronCores.

Per core (one batch element b):
  rgb = x[:1024], depth = x[1024:]
  qkv_m = rgb/depth @ W_m + b_m          (H=8 heads, D=64)
  rgb_out   = softmax(q_dep k_rgb^T / 8) v_rgb   -> out tokens    0..1023
  depth_out = softmax(q_rgb k_dep^T / 8) v_dep   -> out tokens 1024..2047
  out = concat @ W_proj + b_proj

v3 design: the kernel is ACT-bound (exp of 16.8M scores ~ 137us on the
scalar engine, the only engine with Exp).  Everything is braided around a
continuously-fed ACT stream:
  - flat slot loop u=0..127 over (d,g,qh,c): scores emitted one slot
    ahead of PV; deferred QKV/V/proj work popped into PE idle slots.
  - all GEMMs bf16 (fp32 accumulate in PSUM): FWL weight loads, halved
    SBUF/DMA.  x is cast to bf16 and transposed by the DMA XBAR
    (dma_start_transpose, 14ns/tile) - no PE/DVE transpose cost.
  - weight loads are j/k/g-sliced, need-ordered, spread across the three
    DMA rings (sync / gpsimd / scalar) so the ramp is short and the bulk
    streams during the braid.
  - PSUM: scores [128,1024] x2 + PV [65,512] x2 + other [128,512] x2 = 8.
  - softmax runs unnormalized; PV lhsT carries a ones column so Z lands
    in psum row 64; normalize at evac via reciprocal_approx_fast +
    gpsimd partition broadcast.
"""
import numpy as np

import concourse.bass as bass
import concourse.mybir as mybir
import concourse.tile as tile
from concourse import bacc
from concourse.bass_utils import run_bass_kernel_spmd
from concourse.bass_interp import get_hw_module

f32 = mybir.dt.float32
bf16 = mybir.dt.bfloat16
AF = mybir.ActivationFunctionType
Alu = mybir.AluOpType

N_CORES = 8
B, N, C = 8, 2048, 512
H, D, L = 8, 64, 1024
SCALE = float(D) ** -0.5


def _emit(nc, tc, x, w_qkv, b_qkv, w_proj, b_proj, out):
    with (
        tc.tile_pool(name="persist", bufs=1) as persist,
        tc.tile_pool(name="work", bufs=1) as work,
        tc.tile_pool(name="ps", bufs=1, space="PSUM") as ps,
    ):
        # ---------------- persistent tensors ----------------
        # xT[p, t, k, tok] = x[t*128+tok, k*128+p];  t = m*8+c
        xT = persist.tile([128, 16, 4, 128], bf16, name="xT")
        qkT = [persist.tile([128, 8, 1024], bf16, name=f"qkT{m}") for m in range(2)]
        vaug = [persist.tile([128, 8, 8, 65], bf16, name=f"vaug{m}") for m in range(2)]
        oT = [persist.tile([128, 4, 1024], bf16, name=f"oT{d}") for d in range(2)]
        wqk = [persist.tile([128, 4, 8, 128], bf16, name=f"wqk{m}") for m in range(2)]
        wv = [persist.tile([128, 4, 512], bf16, name=f"wv{m}") for m in range(2)]
        wp = persist.tile([128, 4, 512], bf16, name="wp")

        bqk = []
        bv = []
        for m in range(2):
            bq = persist.tile([128, 8], f32, name=f"bqk{m}")
            nc.gpsimd.dma_start(out=bq[:, :],
                                in_=b_qkv[m][0:1024].rearrange("(t p) -> p t", p=128))
            bqk.append(bq)
            bv1 = persist.tile([1, 512], f32, name=f"bv1_{m}")
            nc.gpsimd.dma_start(out=bv1[:, :],
                                in_=b_qkv[m][1024:1536].rearrange("(o f) -> o f", o=1))
            bvm = persist.tile([128, 512], f32, name=f"bv{m}")
            nc.gpsimd.partition_broadcast(bvm[:, :], bv1[:, :])
            bv.append(bvm)
        bp1 = persist.tile([1, 512], f32, name="bp1")
        nc.gpsimd.dma_start(out=bp1[:, :], in_=b_proj.rearrange("(o f) -> o f", o=1))
        bp = persist.tile([128, 512], f32, name="bp")
        nc.gpsimd.partition_broadcast(bp[:, :], bp1[:, :])

        ones64 = persist.tile([128, 64], bf16, name="ones64")
        nc.gpsimd.memset(ones64[:, :], 1.0)
        for m in range(2):
            nc.vector.tensor_copy(
                vaug[m][:, :, :, 64:65],
                ones64.rearrange("p (c h o) -> p c h o", c=8, h=8))

        # ---------------- weight loads (sliced, need-ordered) ----------
        # qeng: which DMA ring issues the slice's HBM->SBUF copy.
        def load_wqk(m, j, qeng):
            """one [512, 128] f32 slice of W_qkv -> wqk[m][:, :, j, :] bf16."""
            st = work.tile([128, 4, 128], f32, name=f"wst{m}_{j}",
                           tag="wst", bufs=3)
            qeng.dma_start(out=st[:, :, :],
                           in_=w_qkv[m][:, j * 128:(j + 1) * 128].rearrange(
                               "(k p) f -> p k f", p=128))
            nc.vector.tensor_copy(wqk[m][:, :, j, :], st[:, :, :])

        def load_wv(m, qeng):
            st = work.tile([128, 4, 512], f32, name=f"wvst{m}",
                           tag="wvst", bufs=1)
            qeng.dma_start(out=st[:, :, :],
                           in_=w_qkv[m][:, 1024:1536].rearrange(
                               "(k p) f -> p k f", p=128))
            nc.vector.tensor_copy(wv[m][:, :, :], st[:, :, :])

        def load_wp(g, qeng):
            st = work.tile([128, 512], f32, name=f"wpst{g}", tag="wst2", bufs=2)
            qeng.dma_start(out=st[:, :],
                           in_=w_proj[g * 128:(g + 1) * 128, :].rearrange(
                               "(o p) f -> p (o f)", p=128))
            nc.vector.tensor_copy(wp[:, g, :], st[:, :])

        # ---------------- emission helpers ----------------
        def emit_xtile(m, c):
            """DMA one [128,512] f32 x tile, cast to bf16, XBAR-transpose."""
            t = m * 8 + c
            xsrc = work.tile([128, 512], f32, name=f"xs{t}", tag="xsrc", bufs=3)
            nc.sync.dma_start(out=xsrc[:, :], in_=x[t * 128:(t + 1) * 128, :])
            x16 = work.tile([128, 512], bf16, name=f"x16_{t}", tag="x16", bufs=3)
            nc.vector.tensor_copy(x16[:, :], xsrc[:, :])
            nc.sync.dma_start_transpose(xT[:, t, :, :], x16[:, :])

        def emit_qk(m, j, th):
            """q/k features j*128.. for tokens th*512.. of stream m."""
            q_ps = ps.tile([128, 512], f32, name=f"qk{m}_{j}_{th}",
                           tag="oth", bufs=2)
            t0 = m * 8 + th * 4
            for k in range(4):
                nc.tensor.matmul(
                    q_ps[:, :],
                    wqk[m][:, k, j, :],
                    xT[:, t0:t0 + 4, k, :],
                    start=(k == 0), stop=(k == 3))
            nc.vector.tensor_scalar(
                out=qkT[m][:, j, th * 512:(th + 1) * 512], in0=q_ps[:, :],
                scalar1=bqk[m][:, j:j + 1], scalar2=None, op0=Alu.add)

        def emit_v(m, c):
            """v for token block c of stream m -> vaug[m][:, c, :, 0:64]."""
            v_ps = ps.tile([128, 512], f32, name=f"v{m}_{c}",
                           tag="oth", bufs=2)
            for k in range(4):
                nc.tensor.matmul(
                    v_ps[:, :],
                    xT[:, m * 8 + c, k, :],
                    wv[m][:, k, :],
                    start=(k == 0), stop=(k == 3))
            nc.vector.tensor_tensor(
                out=vaug[m][:, c, :, 0:64],
                in0=v_ps.rearrange("p (h d) -> p h d", h=8),
                in1=bv[m].rearrange("p (h d) -> p h d", h=8),
                op=Alu.add)

        def emit_proj(d, tt):
            """project token block tt of direction d and store to HBM."""
            pj_ps = ps.tile([128, 512], f32, name=f"pj{d}_{tt}",
                            tag="oth", bufs=2)
            for g in range(4):
                nc.tensor.matmul(
                    pj_ps[:, :],
                    oT[d][:, g, tt * 128:(tt + 1) * 128],
                    wp[:, g, :],
                    start=(g == 0), stop=(g == 3))
            ost = work.tile([128, 512], f32, name=f"ost{d}_{tt}",
                            tag="ost", bufs=3)
            nc.vector.tensor_tensor(out=ost[:, :], in0=pj_ps[:, :],
                                    in1=bp[:, :], op=Alu.add)
            nc.sync.dma_start(
                out=out[d * 1024 + tt * 128:d * 1024 + (tt + 1) * 128, :],
                in_=ost[:, :])

        # Deferred PE work: keyed closures.  need(key) force-emits a
        # producer before its consumer; fill(n) drains in priority order.
        pending = {}
        order = []

        def defer(key, fn):
            pending[key] = fn
            order.append(key)

        def need(*keys):
            for k in keys:
                fn = pending.pop(k, None)
                if fn:
                    fn()

        def fill(n=1):
            while n > 0 and order:
                k = order.pop(0)
                fn = pending.pop(k, None)
                if fn:
                    fn()
                    n -= 1

        # ---------------- prologue (PE-free work + g0 seeds) ----------
        with nc.named_scope("ramp"):
            # exp table preload first on the scalar ring
            scr = persist.tile([1, 128], f32, name="scr")
            nc.gpsimd.memset(scr[:, :], 0.0)
            nc.scalar.activation(scr[:, :], scr[:, :], AF.Exp)
            # ramp-critical weight slices on the gpsimd ring
            load_wqk(0, 4, nc.gpsimd)      # k features g0, rgb
            load_wqk(1, 0, nc.gpsimd)      # q features g0, depth
            load_wv(0, nc.gpsimd)
            # late bulk on the scalar ring (ACT idle during ramp)
            for g in range(1, 4):
                load_wqk(0, 4 + g, nc.gpsimd)
                load_wqk(1, g, nc.scalar)
            for g in range(4):
                load_wqk(1, 4 + g, nc.scalar)
                load_wqk(0, g, nc.scalar)
            load_wv(1, nc.scalar)
            for g in range(4):
                load_wp(g, nc.scalar)
            # all x tiles: DMA + cast + XBAR transpose (no PE work)
            for c in range(4):
                emit_xtile(0, c)
            for c in range(4):
                emit_xtile(1, c)
            for c in range(4, 8):
                emit_xtile(0, c)
            for c in range(4, 8):
                emit_xtile(1, c)
            emit_qk(0, 4, 0)
            emit_qk(1, 0, 0)
            emit_v(0, 0)

        # deferred PE work in first-use order
        defer(("qk", 0, 4, 1), lambda: emit_qk(0, 4, 1))
        for c in range(1, 4):
            defer(("v", 0, c), lambda c=c: emit_v(0, c))
        defer(("qk", 1, 0, 1), lambda: emit_qk(1, 0, 1))
        for c in range(4, 8):
            defer(("v", 0, c), lambda c=c: emit_v(0, c))
        for g in range(1, 4):
            defer(("qk", 0, 4 + g, 0), lambda g=g: emit_qk(0, 4 + g, 0))
            defer(("qk", 0, 4 + g, 1), lambda g=g: emit_qk(0, 4 + g, 1))
            defer(("qk", 1, g, 0), lambda g=g: emit_qk(1, g, 0))
            defer(("qk", 1, g, 1), lambda g=g: emit_qk(1, g, 1))
        for g in range(4):                                      # d1 lhsT
            defer(("qk", 1, 4 + g, 0), lambda g=g: emit_qk(1, 4 + g, 0))
            defer(("qk", 1, 4 + g, 1), lambda g=g: emit_qk(1, 4 + g, 1))
        for g in range(4):                                      # d1 rhs
            defer(("qk", 0, g, 0), lambda g=g: emit_qk(0, g, 0))
            defer(("qk", 0, g, 1), lambda g=g: emit_qk(0, g, 1))
        for c in range(8):                                      # d1 PV v
            defer(("v", 1, c), lambda c=c: emit_v(1, c))

        # ---------------- main braid: flat slot loop ----------------
        # slot u = (d, g, qh, c); scores one slot ahead of PV.
        def slot(u):
            c = u % 8
            qh = (u // 8) % 2
            g = (u // 16) % 4
            d = u // 64
            return d, g, qh, c

        state = {}          # (d,g,qh) -> dict(oT_ps, exps)

        def emit_scores(u):
            d, g, qh, c = slot(u)
            qm, kvm = 1 - d, d
            need(("qk", kvm, 4 + g, c // 4), ("qk", qm, g, qh))
            s_ps = ps.tile([128, 1024], f32, name=f"s{u}", tag="sc", bufs=2)
            for hh in range(2):
                pb = hh * 64
                nc.tensor.matmul(
                    s_ps[:, hh * 512:(hh + 1) * 512],
                    qkT[kvm][pb:pb + 64, 4 + g, c * 128:(c + 1) * 128],
                    qkT[qm][pb:pb + 64, g, qh * 512:(qh + 1) * 512],
                    start=True, stop=True)
            exp_t = work.tile([128, 1024], bf16, name=f"e{u}", tag="exp", bufs=6)
            nc.scalar.activation(exp_t[:, :], s_ps[:, :], AF.Exp, scale=SCALE)
            st = state.setdefault((d, g, qh), {"exps": {}})
            st["exps"][c] = exp_t

        def emit_pv(u):
            d, g, qh, c = slot(u)
            kvm = d
            need(("v", kvm, c))
            st = state[(d, g, qh)]
            if c == 0:
                st["oT_ps"] = [
                    ps.tile([65, 512], f32, name=f"o{u}_{hh}",
                            tag=f"pv{hh}", bufs=1)
                    for hh in range(2)
                ]
            exp_t = st["exps"].pop(c)
            for hh in range(2):
                nc.tensor.matmul(
                    st["oT_ps"][hh][:, :],
                    vaug[kvm][:, c, 2 * g + hh, :],
                    exp_t[:, hh * 512:(hh + 1) * 512],
                    start=(c == 0), stop=(c == 7))
            if c == 7:
                emit_evac(d, g, qh, st["oT_ps"])
                del state[(d, g, qh)]

        def emit_evac(d, g, qh, oT_ps):
            # copy to sbuf, Z -> 1/Z -> broadcast -> normalize
            oTs = []
            zst = work.tile([8, 128], f32, name=f"z{d}{g}{qh}", tag="zst", bufs=2)
            for hh in range(2):
                ot = work.tile([65, 512], f32, name=f"oTs{d}{g}{qh}{hh}",
                               tag="oTs", bufs=4)
                nc.vector.tensor_copy(ot[:, :], oT_ps[hh][:, :])
                oTs.append(ot)
                nc.sync.dma_start(
                    out=zst[4 * hh:4 * hh + 4, :],
                    in_=ot[64:65, :].rearrange("o (j t) -> o j t", j=4))
            rst = work.tile([8, 128], f32, name=f"r{d}{g}{qh}", tag="rst", bufs=2)
            nc.vector.reciprocal_approx_fast(rst[:, :], zst[:, :])
            for hh in range(2):
                rz = work.tile([1, 512], f32, name=f"rz{d}{g}{qh}{hh}",
                               tag="rz", bufs=2)
                nc.sync.dma_start(
                    out=rz[0:1, :].rearrange("o (j t) -> o j t", j=4),
                    in_=rst[4 * hh:4 * hh + 4, :])
                rzb = work.tile([64, 512], f32, name=f"rzb{d}{g}{qh}{hh}",
                                tag="rzb", bufs=2)
                nc.gpsimd.partition_broadcast(rzb[:, :], rz[:, :])
                nc.vector.tensor_tensor(
                    out=oT[d][hh * 64:hh * 64 + 64, g,
                              qh * 512:(qh + 1) * 512],
                    in0=oTs[hh][0:64, :], in1=rzb[:, :],
                    op=Alu.mult)

        with nc.named_scope("braid"):
            for u in range(130):
                if u == 64:
                    for tt in range(8):
                        defer(("proj", 0, tt), lambda tt=tt: emit_proj(0, tt))
                if u < 128:
                    emit_scores(u)
                if u >= 2:
                    emit_pv(u - 2)
                fill(1)

        # ---------------- tail ----------------
        with nc.named_scope("tail"):
            fill(len(order))
            for tt in range(8):
                emit_proj(1, tt)


def build_module():
    nc = bacc.Bacc("TRN2", target_bir_lowering=False, debug=False,
                   num_devices=N_CORES)
    x = nc.dram_tensor("x", [N, C], f32, kind="ExternalInput").ap()
    w_rgb = nc.dram_tensor("w_rgb", [C, 3 * C], f32, kind="ExternalInput").ap()
    b_rgb = nc.dram_tensor("b_rgb", [3 * C], f32, kind="ExternalInput").ap()
    w_dep = nc.dram_tensor("w_dep", [C, 3 * C], f32, kind="ExternalInput").ap()
    b_dep = nc.dram_tensor("b_dep", [3 * C], f32, kind="ExternalInput").ap()
    w_proj = nc.dram_tensor("w_proj", [C, C], f32, kind="ExternalInput").ap()
    b_proj = nc.dram_tensor("b_proj", [C], f32, kind="ExternalInput").ap()
    out = nc.dram_tensor("out", [N, C], f32, kind="ExternalOutput").ap()

    with tile.TileContext(nc) as tc:
        _emit(nc, tc, x, [w_rgb, w_dep], [b_rgb, b_dep], w_proj, b_proj, out)
    nc.compile()
    nc.m = get_hw_module(nc.m)
    return nc


_NC_CACHE = None


def kernel(x, W_rgb_qkv, b_rgb_qkv, W_depth_qkv, b_depth_qkv, W_proj, b_proj):
    global _NC_CACHE
    if _NC_CACHE is None:
        _NC_CACHE = build_module()
    nc = _NC_CACHE

    x = np.ascontiguousarray(np.asarray(x, dtype=np.float32))
    shared = {
        "w_rgb": np.ascontiguousarray(np.asarray(W_rgb_qkv, np.float32)),
        "b_rgb": np.ascontiguousarray(np.asarray(b_rgb_qkv, np.float32)),
        "w_dep": np.ascontiguousarray(np.asarray(W_depth_qkv, np.float32)),
        "b_dep": np.ascontiguousarray(np.asarray(b_depth_qkv, np.float32)),
        "w_proj": np.ascontiguousarray(np.asarray(W_proj, np.float32)),
        "b_proj": np.ascontiguousarray(np.asarray(b_proj, np.float32)),
    }
    in_maps = [{"x": x[i], **shared} for i in range(N_CORES)]
    res = run_bass_kernel_spmd(nc, in_maps, core_ids=list(range(N_CORES)))
    return np.stack([res.results[i]["out"] for i in range(N_CORES)], axis=0)

